# revision 10
# baseline (speedup 1.0000x reference)
"""Distributed 3-layer GAT kernel for Trainium2 (8 NeuronCores).

Strategy (dst-sharded edges, node-sharded dense):
  - Destination nodes are assigned to 80 (core, block) bins by a greedy
    balance on in-degree, so every bin carries ~E/80 edges and the per-core
    gather row counts are uniform (the HBM row-gather at ~14 ns/row is the
    kernel's hard floor; balanced bins minimize the max-core row count).
  - Each core owns all edges whose destination bin lives on it, so the
    per-destination softmax needs no cross-core reduction.
  - Layer 1's dense projection (contraction dim = 96) is REPLICATED: every
    core computes the full 10240-row z table locally from the (tiny,
    replicated) input, eliminating layer 1's AllGather entirely.
  - Layers 2/3: each core computes z for its own rows, then chunked
    AllGathers replicate the table, overlapping the previous layer's edge
    processing.
  - Tables are [z fp8 | a_src bf16] rows (1040/770 bytes, u8 tensors +
    bitcast views): halves AllGather volume and HBM write traffic (less
    contention with the random gathers). The fp8 cast is a DVE copy at
    table-write time (once per row), and the gs multiply consumes fp8
    directly via a mixed-dtype DVE op -- no per-edge convert. (Casting
    during SWDGE DMA was tried and is pathologically slow.)
  - Edge chunks (128 edges) gather their source rows via indirect DMA into a
    per-block tile; one-hot(dst-slot) matmuls on the PE accumulate softmax
    denominators and scatter-add messages into PSUM. The alpha chain is
    packed per block ([128, M*H] ops) instead of per chunk.
  - Softmax is computed without the segment-max shift (logits are bounded
    for this model); the 1e-16 eps matches the reference.
"""

import sys

sys.path.insert(0, "/opt/trn_rl_repo")

import heapq

import numpy as np

# Problem constants (hardcoded per contract)
N = 10000
E = 160000
SEQ = 96
HID = 128
HEADS = 8
OUT = 768
HC = HID * HEADS  # 1024

NCORES = 8
NB = 10      # 128-node destination blocks per core
NPC = NB * 128  # padded nodes per core (1280)
NPAD = NPC
P = 128
NBINS = NCORES * NB
GR = 256     # rows per AllGather group (2 node blocks)

# table row layouts (bytes): [z fp8 (FO) | a_src bf16 (2H)]
ROWB = [HC + 2 * HEADS, HC + 2 * HEADS, OUT + 2]  # 1040, 1040, 770

LAST_RESULT = None


def _edge_prep(edge_index, edge_weight):
    """Balanced dst binning + per-core chunked edge arrays.

    Returns (MB, offs, CHT, metas, node_core, node_loc) where node_loc is the
    0..1279 row of each node within its core (block*128 + slot).
    """
    src = edge_index[0].astype(np.int64)
    dst = edge_index[1].astype(np.int64)
    indeg = np.bincount(dst, minlength=N)

    # greedy: highest-degree nodes first, into the least-loaded non-full bin
    order = np.argsort(-indeg, kind="stable")
    bin_cnt = np.zeros(NBINS, np.int64)
    node_bin = np.empty(N, np.int64)
    heap = [(0, b) for b in range(NBINS)]
    heapq.heapify(heap)
    for n in order:
        while True:
            load, b = heapq.heappop(heap)
            if bin_cnt[b] < P:
                break
        node_bin[n] = b
        bin_cnt[b] += 1
        if bin_cnt[b] < P:
            heapq.heappush(heap, (load + int(indeg[n]), b))
    node_core = node_bin // NB
    node_block = node_bin % NB
    node_slot = np.empty(N, np.int64)
    for b in range(NBINS):
        nodes = np.nonzero(node_bin == b)[0]
        node_slot[nodes] = np.arange(len(nodes))
    node_loc = node_block * P + node_slot

    # table row ids of each edge's source
    srank, sloc = node_core[src], node_loc[src]
    row_r = srank * NPAD + sloc                                   # rank-major (L1)
    row_g = (sloc // GR) * (NCORES * GR) + srank * GR + sloc % GR  # group-major (L2/3)

    dcore, dblock, dslot = node_core[dst], node_block[dst], node_slot[dst]

    cnt = np.zeros((NCORES, NB), np.int64)
    np.add.at(cnt, (dcore, dblock), 1)
    MB = np.maximum(1, (cnt.max(axis=0) + P - 1) // P).astype(np.int64)
    offs = np.concatenate([[0], np.cumsum(MB)]).astype(np.int64)
    CHT = int(MB.sum())

    metas = []
    for c in range(NCORES):
        srm = np.zeros((P, CHT), np.int32)
        srmG = np.zeros((P, CHT), np.int32)
        dm = np.full((P, CHT), 999.0, np.float32)
        ewm = np.zeros((P, CHT), np.float32)
        for b in range(NB):
            sel = np.nonzero((dcore == c) & (dblock == b))[0]
            k = len(sel)
            lanes = np.arange(k) % P
            cols = offs[b] + np.arange(k) // P
            srm[lanes, cols] = row_r[sel]
            srmG[lanes, cols] = row_g[sel]
            dm[lanes, cols] = dslot[sel].astype(np.float32)
            ewm[lanes, cols] = edge_weight[sel]
        metas.append(dict(srcrow=srm, srcrowG=srmG, dstmod=dm, ew=ewm))
    return MB, offs, CHT, metas, node_core, node_loc


def _build_program(MB, offs, CHT, sim_single_core=False):
    from concourse import bass, bacc, mybir, tile
    from concourse.masks import make_identity

    f32 = mybir.dt.float32
    bf = mybir.dt.bfloat16
    f8 = mybir.dt.float8e4
    u8 = mybir.dt.uint8
    i32 = mybir.dt.int32
    AT = mybir.ActivationFunctionType
    OP = mybir.AluOpType

    ndev = 1 if sim_single_core else NCORES
    nc = bacc.Bacc(None, target_bir_lowering=False, debug=False,
                   num_devices=ndev, num_swdge_queues=4)

    # ---------------- I/O ----------------
    xT_t = nc.dram_tensor("xT", [SEQ, NCORES * NPAD], bf, kind="ExternalInput")
    xTown_t = nc.dram_tensor("xTown", [SEQ, NPAD], bf, kind="ExternalInput")
    W_t = [
        nc.dram_tensor("W1", [SEQ, HC], bf, kind="ExternalInput"),
        nc.dram_tensor("W2", [HC, HC], bf, kind="ExternalInput"),
        nc.dram_tensor("W3", [HC, OUT], bf, kind="ExternalInput"),
    ]
    asb_t = [
        nc.dram_tensor("asb1", [P, HC], bf, kind="ExternalInput"),
        nc.dram_tensor("asb2", [P, HC], bf, kind="ExternalInput"),
        nc.dram_tensor("asb3", [P, OUT], bf, kind="ExternalInput"),
    ]
    adb_t = [
        nc.dram_tensor("adb1", [P, HC], bf, kind="ExternalInput"),
        nc.dram_tensor("adb2", [P, HC], bf, kind="ExternalInput"),
        nc.dram_tensor("adb3", [P, OUT], bf, kind="ExternalInput"),
    ]
    ceb_t = [
        nc.dram_tensor("ceb1", [P, HEADS], bf, kind="ExternalInput"),
        nc.dram_tensor("ceb2", [P, HEADS], bf, kind="ExternalInput"),
        nc.dram_tensor("ceb3", [P, 1], bf, kind="ExternalInput"),
    ]
    bb_t = [
        nc.dram_tensor("bb1", [P, HC], f32, kind="ExternalInput"),
        nc.dram_tensor("bb2", [P, HC], f32, kind="ExternalInput"),
        nc.dram_tensor("bb3", [P, OUT], f32, kind="ExternalInput"),
    ]
    srcrow_t = nc.dram_tensor("srcrow", [P, CHT], i32, kind="ExternalInput")
    srcrowG_t = nc.dram_tensor("srcrowG", [P, CHT], i32, kind="ExternalInput")
    dstmod_t = nc.dram_tensor("dstmod", [P, CHT], bf, kind="ExternalInput")
    ew_t = nc.dram_tensor("ewt", [P, CHT], bf, kind="ExternalInput")
    out_t = nc.dram_tensor("out", [NPAD, OUT], f32, kind="ExternalOutput")

    # layer configs: (K_in, FO, H, C, relu)
    LCFG = [
        (SEQ, HC, HEADS, HID, True),
        (HC, HC, HEADS, HID, True),
        (HC, OUT, 1, OUT, False),
    ]

    with tile.TileContext(nc) as tc:
        with (
            tc.tile_pool(name="const", bufs=1) as cpool,
            tc.tile_pool(name="dram", bufs=1, space="DRAM") as dpool,
            tc.tile_pool(name="work", bufs=2) as wpool,
            tc.tile_pool(name="gat", bufs=2) as gpool,
            tc.tile_pool(name="pbig", bufs=2, space="PSUM") as pbig,
            tc.tile_pool(name="psmall", bufs=2, space="PSUM") as psmall,
        ):
            # ---------------- constants ----------------
            ident = cpool.tile([P, P], bf, name="ident", tag="ident")
            make_identity(nc, ident[:])
            iota_i = cpool.tile([P, P], i32, name="iota_i", tag="iota_i")
            nc.gpsimd.iota(iota_i[:], pattern=[[1, P]], base=0, channel_multiplier=0)
            iota_f = cpool.tile([P, P], bf, name="iota_f", tag="iota_f")
            nc.vector.tensor_copy(iota_f[:], iota_i[:])

            xT_sb = cpool.tile([SEQ, NCORES * NPAD], bf, name="xT_sb", tag="xT_sb")
            nc.sync.dma_start(xT_sb[:], xT_t[:])
            xTown_sb = cpool.tile([SEQ, NPAD], bf, name="xTown_sb", tag="xTown_sb")
            nc.sync.dma_start(xTown_sb[:], xTown_t[:])

            srcrow_sb = cpool.tile([P, CHT], i32, name="srcrow_sb", tag="srcrow_sb")
            nc.sync.dma_start(srcrow_sb[:], srcrow_t[:])
            srcrowG_sb = cpool.tile([P, CHT], i32, name="srcrowG_sb", tag="srcrowG_sb")
            nc.sync.dma_start(srcrowG_sb[:], srcrowG_t[:])
            dstmod_sb = cpool.tile([P, CHT], bf, name="dstmod_sb", tag="dstmod_sb")
            nc.sync.dma_start(dstmod_sb[:], dstmod_t[:])
            ew_sb = cpool.tile([P, CHT], bf, name="ew_sb", tag="ew_sb")
            nc.sync.dma_start(ew_sb[:], ew_t[:])

            # ---------------- internal DRAM ----------------
            ci, tb = [None], []
            for li in range(3):
                tb.append(
                    dpool.tile([NCORES * NPAD, ROWB[li]], u8,
                               name=f"tb{li}", tag=f"tb{li}")
                )
            for li in (1, 2):
                ci.append(
                    dpool.tile([NPAD, ROWB[li]], u8, name=f"ci{li}", tag=f"ci{li}")
                )

            def stats_from_sbuf(zb, coef, H, C, FO):
                tmp = wpool.tile([P, FO], bf, name="tmp", tag="stat_tmp")
                nc.vector.tensor_mul(tmp[:], zb[:], coef[:])
                red = wpool.tile([P, H], f32, name="red", tag="red", bufs=4)
                nc.vector.tensor_reduce(
                    out=red[:],
                    in_=tmp[:].rearrange("p (h c) -> p h c", c=C),
                    axis=mybir.AxisListType.X,
                    op=OP.add,
                )
                redb = wpool.tile([P, H], bf, name="redb", tag="redb", bufs=4)
                nc.vector.tensor_copy(redb[:], red[:])
                return redb

            def load_layer_params(li, H, FO, K_in):
                nk = (K_in + P - 1) // P
                W_l = []
                for kc in range(nk):
                    k0, k1 = kc * P, min(K_in, kc * P + P)
                    wt = cpool.tile([k1 - k0, FO], bf, name="wt", tag=f"w_{kc}")
                    nc.sync.dma_start(wt[:], W_t[li][k0:k1, :])
                    W_l.append(wt)
                a_s_b = cpool.tile([P, FO], bf, name="a_s_b", tag="asb")
                nc.sync.dma_start(a_s_b[:], asb_t[li][:])
                a_d_b = cpool.tile([P, FO], bf, name="a_d_b", tag="adb")
                nc.sync.dma_start(a_d_b[:], adb_t[li][:])
                ce_b = cpool.tile([P, H], bf, name="ce_b", tag="ceb", bufs=2)
                nc.sync.dma_start(ce_b[:], ceb_t[li][:])
                bb_b = cpool.tile([P, FO], f32, name="bb_b", tag="bb", bufs=2)
                nc.sync.dma_start(bb_b[:], bb_t[li][:])
                return W_l, a_s_b, a_d_b, ce_b, bb_b

            # ---------------- layer 1: replicated full-table sweep ----------
            W1_l, as1_b, ad1_b, ce1_b, bb1_b = load_layer_params(0, HEADS, HC, SEQ)
            adall1 = wpool.tile([P, NB * HEADS], bf, name="adall1", tag="adall", bufs=2)
            for gb in range(NCORES * NB):
                z_ps = pbig.tile([P, HC], f32, name="z_ps", tag="big")
                for j in range(2):
                    j0, j1 = j * 512, (j + 1) * 512
                    nc.tensor.matmul(
                        out=z_ps[:, j0:j1],
                        lhsT=xT_sb[:, gb * P : (gb + 1) * P],
                        rhs=W1_l[0][:, j0:j1],
                        start=True, stop=True,
                    )
                zb = wpool.tile([P, HC], bf, name="zb", tag="zb")
                nc.vector.tensor_copy(zb[:], z_ps[:])
                z8 = wpool.tile([P, HC], f8, name="z8", tag="z8")
                nc.vector.tensor_copy(z8[:], z_ps[:])
                nc.sync.dma_start(
                    tb[0][gb * P : (gb + 1) * P, 0:HC].bitcast(f8), z8[:]
                )
                redb = stats_from_sbuf(zb, as1_b, HEADS, HID, HC)
                nc.sync.dma_start(
                    tb[0][gb * P : (gb + 1) * P, HC : HC + 2 * HEADS].bitcast(bf),
                    redb[:],
                )
            # own-rows mini-sweep for a_dst (rank-independent: uses own input)
            for nb in range(NB):
                z_ps = pbig.tile([P, HC], f32, name="z_ps", tag="big")
                for j in range(2):
                    j0, j1 = j * 512, (j + 1) * 512
                    nc.tensor.matmul(
                        out=z_ps[:, j0:j1],
                        lhsT=xTown_sb[:, nb * P : (nb + 1) * P],
                        rhs=W1_l[0][:, j0:j1],
                        start=True, stop=True,
                    )
                zb = wpool.tile([P, HC], bf, name="zb", tag="zb")
                nc.vector.tensor_copy(zb[:], z_ps[:])
                redb = stats_from_sbuf(zb, ad1_b, HEADS, HID, HC)
                nc.vector.tensor_copy(
                    adall1[:, nb * HEADS : (nb + 1) * HEADS], redb[:]
                )

            # ---------------- helpers ------------------------------------
            def dense_block(li, nb, f_in, W_l, a_s_b, a_d_b, adall):
                K_in, FO, H, C, relu = LCFG[li]
                nk = (K_in + P - 1) // P
                nj = (FO + 511) // 512
                lhsTs = []
                for kc in range(nk):
                    tr_ps = psmall.tile([P, P], bf, name="tr_ps", tag="tr", bufs=1)
                    nc.tensor.transpose(
                        out=tr_ps[:],
                        in_=f_in[:, kc * P : (kc + 1) * P],
                        identity=ident[:],
                    )
                    lt = wpool.tile([P, P], bf, name="lt", tag="lt", bufs=10)
                    nc.vector.tensor_copy(lt[:], tr_ps[:])
                    lhsTs.append(lt[:])
                z_ps = pbig.tile([P, FO], f32, name="z_ps", tag="big")
                for j in range(nj):
                    j0, j1 = j * 512, min(FO, (j + 1) * 512)
                    for kc in range(nk):
                        nc.tensor.matmul(
                            out=z_ps[:, j0:j1],
                            lhsT=lhsTs[kc],
                            rhs=W_l[kc][:, j0:j1],
                            start=(kc == 0),
                            stop=(kc == nk - 1),
                        )
                zb = wpool.tile([P, FO], bf, name="zb", tag="zb")
                nc.vector.tensor_copy(zb[:], z_ps[:])
                z8 = wpool.tile([P, FO], f8, name="z8", tag="z8")
                nc.vector.tensor_copy(z8[:], z_ps[:])
                nc.sync.dma_start(
                    ci[li][nb * P : (nb + 1) * P, 0:FO].bitcast(f8), z8[:]
                )
                redb = stats_from_sbuf(zb, a_s_b, H, C, FO)
                nc.sync.dma_start(
                    ci[li][nb * P : (nb + 1) * P, FO : FO + 2 * H].bitcast(bf),
                    redb[:],
                )
                redd = stats_from_sbuf(zb, a_d_b, H, C, FO)
                nc.vector.tensor_copy(adall[:, nb * H : (nb + 1) * H], redd[:])

            def ag(li, r0, r1):
                g8 = (r0 // GR) * NCORES * GR
                if sim_single_core:
                    nc.gpsimd.dma_start(
                        tb[li][g8 : g8 + (r1 - r0), :], ci[li][r0:r1, :]
                    )
                else:
                    nc.gpsimd.collective_compute(
                        "AllGather",
                        OP.bypass,
                        replica_groups=[list(range(NCORES))],
                        ins=[ci[li][r0:r1, :].opt()],
                        outs=[tb[li][g8 : g8 + NCORES * (r1 - r0), :].opt()],
                    )

            def agg_block(li, nb, srcrow, ce_b, bb_b, adall):
                K_in, FO, H, C, relu = LCFG[li]
                RB = ROWB[li]
                nj = (FO + 511) // 512
                M = int(MB[nb])
                c0 = int(offs[nb])

                g_all = gpool.tile([P, M * RB], u8, name="g_all", tag="g", bufs=3)
                for m in range(M):
                    nc.gpsimd.indirect_dma_start(
                        out=g_all[:, m * RB : (m + 1) * RB],
                        out_offset=None,
                        in_=tb[li][:],
                        in_offset=bass.IndirectOffsetOnAxis(
                            ap=srcrow[:, c0 + m : c0 + m + 1], axis=0
                        ),
                    )
                g3 = g_all[:].rearrange("p (m r) -> p m r", r=RB)

                oh_all = gpool.tile([P, M * P], bf, name="oh_all", tag="oh", bufs=3)
                nc.vector.tensor_tensor(
                    out=oh_all[:].rearrange("p (m j) -> p m j", j=P),
                    in0=dstmod_sb[:, c0 : c0 + M].unsqueeze(2).to_broadcast([P, M, P]),
                    in1=iota_f[:].unsqueeze(1).to_broadcast([P, M, P]),
                    op=OP.is_equal,
                )

                ad_ps = psmall.tile([P, M * H], f32, name="ad_ps", tag="adp", bufs=2)
                for m in range(M):
                    trB = psmall.tile([P, P], bf, name="trB", tag="tr", bufs=1)
                    nc.tensor.transpose(
                        out=trB[:], in_=oh_all[:, m * P : (m + 1) * P],
                        identity=ident[:],
                    )
                    ohB = wpool.tile([P, P], bf, name="ohB", tag="ohB", bufs=3)
                    nc.vector.tensor_copy(ohB[:], trB[:])
                    nc.tensor.matmul(
                        out=ad_ps[:, m * H : (m + 1) * H],
                        lhsT=ohB[:],
                        rhs=adall[:, nb * H : (nb + 1) * H],
                        start=True, stop=True,
                    )

                # packed alpha chain over [P, M*H]
                asrc_ap = g3[:, :, FO : FO + 2 * H].bitcast(bf)
                ewce = wpool.tile([P, M * H], bf, name="ewce", tag="ewce")
                nc.vector.tensor_tensor(
                    out=ewce[:].rearrange("p (m h) -> p m h", h=H),
                    in0=ew_sb[:, c0 : c0 + M].unsqueeze(2).to_broadcast([P, M, H]),
                    in1=ce_b[:].unsqueeze(1).to_broadcast([P, M, H]),
                    op=OP.mult,
                )
                al = wpool.tile([P, M * H], bf, name="al", tag="al")
                nc.vector.tensor_tensor(
                    out=al[:].rearrange("p (m h) -> p m h", h=H),
                    in0=ewce[:].rearrange("p (m h) -> p m h", h=H),
                    in1=asrc_ap,
                    op=OP.add,
                )
                al2 = wpool.tile([P, M * H], bf, name="al2", tag="al2")
                nc.vector.tensor_add(al2[:], al[:], ad_ps[:])
                al3 = wpool.tile([P, M * H], bf, name="al3", tag="al3")
                nc.vector.scalar_tensor_tensor(
                    out=al3[:], in0=al2[:], scalar=0.2, in1=al2[:],
                    op0=OP.mult, op1=OP.max,
                )
                ex_all = wpool.tile([P, M * H], bf, name="ex_all", tag="ex", bufs=2)
                nc.scalar.activation(out=ex_all[:], in_=al3[:], func=AT.Exp)

                # weighted scatter + denominator
                agg_ps = pbig.tile([P, FO], f32, name="agg_ps", tag="big")
                den_ps = psmall.tile([P, H], f32, name="den_ps", tag="den", bufs=1)
                for m in range(M):
                    gs = wpool.tile([P, FO], bf, name="gs", tag="gs", bufs=3)
                    nc.vector.tensor_tensor(
                        out=gs[:].rearrange("p (h c) -> p h c", c=C),
                        in0=g3[:, m, 0:FO].bitcast(f8)
                            .rearrange("p (h c) -> p h c", c=C),
                        in1=ex_all[:, m * H : (m + 1) * H]
                            .unsqueeze(2).to_broadcast([P, H, C]),
                        op=OP.mult,
                    )
                    nc.tensor.matmul(
                        out=den_ps[:],
                        lhsT=oh_all[:, m * P : (m + 1) * P],
                        rhs=ex_all[:, m * H : (m + 1) * H],
                        start=(m == 0), stop=(m == M - 1),
                    )
                    for j in range(nj):
                        j0, j1 = j * 512, min(FO, (j + 1) * 512)
                        nc.tensor.matmul(
                            out=agg_ps[:, j0:j1],
                            lhsT=oh_all[:, m * P : (m + 1) * P],
                            rhs=gs[:, j0:j1],
                            start=(m == 0), stop=(m == M - 1),
                        )

                den_sb = wpool.tile([P, H], f32, name="den_sb", tag="den_sb")
                nc.vector.tensor_scalar_add(den_sb[:], den_ps[:], 1e-16)
                rec = wpool.tile([P, H], f32, name="rec", tag="rec")
                nc.vector.reciprocal(rec[:], den_sb[:])
                o1 = wpool.tile([P, FO], f32, name="o1", tag="o1")
                nc.vector.tensor_tensor(
                    out=o1[:].rearrange("p (h c) -> p h c", c=C),
                    in0=agg_ps[:].rearrange("p (h c) -> p h c", c=C),
                    in1=rec[:].unsqueeze(2).to_broadcast([P, H, C]),
                    op=OP.mult,
                )
                o2 = wpool.tile([P, FO], f32, name="o2", tag="o2")
                nc.vector.tensor_add(o2[:], o1[:], bb_b[:])
                if relu:
                    fnew = wpool.tile([P, FO], bf, name="fnew", tag="fnew")
                    nc.scalar.activation(out=fnew[:], in_=o2[:], func=AT.Relu)
                    return fnew
                nc.sync.dma_start(out_t[nb * P : (nb + 1) * P, :], o2[:])
                return None

            # ---------------- layers 2/3 pipelined over blocks -------------
            prevctx = (0, srcrow_sb, ce1_b, bb1_b, adall1)
            for li in (1, 2):
                K_in, FO, H, C, relu = LCFG[li]
                W_l, a_s_b, a_d_b, ce_b, bb_b = load_layer_params(li, H, FO, K_in)
                adall = wpool.tile([P, NB * H], bf, name="adall", tag="adall", bufs=2)

                for nb in range(NB):
                    pli, psrc, pce, pbb, pad = prevctx
                    fnew = agg_block(pli, nb, psrc, pce, pbb, pad)
                    dense_block(li, nb, fnew, W_l, a_s_b, a_d_b, adall)
                    if nb % 2 == 1:
                        g = (nb - 1) // 2
                        ag(li, g * GR, (g + 1) * GR)

                prevctx = (li, srcrowG_sb, ce_b, bb_b, adall)

            pli, psrc, pce, pbb, pad = prevctx
            for nb in range(NB):
                agg_block(pli, nb, psrc, pce, pbb, pad)

    nc.finalize()
    return nc


def _run_via_pjrt(nc, in_maps):
    """Multi-core run via PJRT without output donation, returning a reusable
    compiled callable for steady-state timing."""
    import jax
    import numpy as _np
    from jax.sharding import Mesh, PartitionSpec
    from jax.experimental.shard_map import shard_map
    from concourse import bass2jax, mybir

    bass2jax.install_neuronx_cc_hook()

    partition_name = nc.partition_id_tensor.name if nc.partition_id_tensor else None
    in_names, out_names, out_avals, zero_outs = [], [], [], []
    for alloc in nc.m.functions[0].allocations:
        if not isinstance(alloc, mybir.MemoryLocationSet):
            continue
        name = alloc.memorylocations[0].name
        if alloc.kind == "ExternalInput":
            if name != partition_name:
                in_names.append(name)
        elif alloc.kind == "ExternalOutput":
            shape = tuple(alloc.tensor_shape)
            dtype = mybir.dt.np(alloc.dtype)
            out_names.append(name)
            out_avals.append(jax.core.ShapedArray(shape, dtype))
            zero_outs.append(_np.zeros(shape, dtype))
    n_params = len(in_names)
    all_in_names = in_names + out_names
    if partition_name is not None:
        all_in_names = all_in_names + [partition_name]

    def _body(*args):
        operands = list(args)
        if partition_name is not None:
            operands.append(bass2jax.partition_id_tensor())
        outs = bass2jax._bass_exec_p.bind(
            *operands,
            out_avals=tuple(out_avals),
            in_names=tuple(all_in_names),
            out_names=tuple(out_names),
            lowering_input_output_aliases=(),
            sim_require_finite=True,
            sim_require_nnan=True,
            nc=nc,
        )
        return tuple(outs)

    n = len(in_maps)
    devices = jax.devices()[:n]
    mesh = Mesh(_np.asarray(devices), ("core",))
    specs = (PartitionSpec("core"),) * (n_params + len(out_names))
    out_specs = (PartitionSpec("core"),) * len(out_names)
    fn = jax.jit(
        shard_map(_body, mesh=mesh, in_specs=specs, out_specs=out_specs,
                  check_rep=False),
        keep_unused=True,
    )
    concat_in = [
        _np.concatenate([_np.asarray(in_maps[c][k]) for c in range(n)], axis=0)
        for k in in_names
    ] + [
        _np.zeros((n * z.shape[0], *z.shape[1:]), z.dtype) for z in zero_outs
    ]
    sharding = jax.sharding.NamedSharding(mesh, PartitionSpec("core"))
    dev_in = [jax.device_put(a, sharding) for a in concat_in]
    out_arrs = fn(*dev_in)
    jax.block_until_ready(out_arrs)
    results = [
        {
            name: _np.asarray(out_arrs[i]).reshape(n, *out_avals[i].shape)[c]
            for i, name in enumerate(out_names)
        }
        for c in range(n)
    ]
    return results, (fn, dev_in)


_BENCH = None


def bench(n_iters=20):
    """Median wall time (ns) of one steady-state invocation of the compiled
    8-core executable with device-resident inputs."""
    import jax, time
    assert _BENCH is not None, "call kernel() first"
    fn, dev_in = _BENCH
    jax.block_until_ready(fn(*dev_in))  # warm
    times = []
    for _ in range(n_iters):
        t0 = time.perf_counter()
        jax.block_until_ready(fn(*dev_in))
        t1 = time.perf_counter()
        times.append(t1 - t0)
    times.sort()
    return times[len(times) // 2] * 1e9


def kernel(**inputs):
    global LAST_RESULT, _BENCH

    x = np.asarray(inputs["x"], np.float32)
    edge_index = np.asarray(inputs["edge_index"], np.int32)
    edge_weight = np.asarray(inputs["edge_weight"], np.float32)

    MB, offs, CHT, metas, node_core, node_loc = _edge_prep(edge_index, edge_weight)
    nc = _build_program(MB, offs, CHT)

    # xT in global table-row (rank-major) order, pad slots zero
    xT = np.zeros((SEQ, NCORES * NPAD), np.float32)
    xT[:, node_core * NPAD + node_loc] = x[0]

    def bcast(v):
        v = np.asarray(v, np.float32).reshape(1, -1)
        return np.ascontiguousarray(np.repeat(v, P, axis=0))

    Ws = [np.asarray(inputs[k], np.float32) for k in ("W1", "W2", "W3")]
    layer_params = []
    for li, (aek, wek, ask, adk, bk, H, C) in enumerate(
        (
            ("ae1", "We1", "as1", "ad1", "b1", HEADS, HID),
            ("ae2", "We2", "as2", "ad2", "b2", HEADS, HID),
            ("ae3", "We3", "as3", "ad3", "b3", 1, OUT),
        )
    ):
        ae = np.asarray(inputs[aek], np.float32)
        We = np.asarray(inputs[wek], np.float32)
        ce = np.array(
            [We[0, h * C : (h + 1) * C] @ ae[h] for h in range(H)], np.float32
        )
        layer_params.append(
            dict(
                asb=bcast(np.asarray(inputs[ask], np.float32).reshape(-1)),
                adb=bcast(np.asarray(inputs[adk], np.float32).reshape(-1)),
                ceb=bcast(ce),
                bb=bcast(np.asarray(inputs[bk], np.float32)),
            )
        )

    import ml_dtypes

    bf16 = ml_dtypes.bfloat16
    in_maps = []
    for c in range(NCORES):
        own = xT[:, c * NPAD : (c + 1) * NPAD]
        m = dict(
            xT=xT.astype(bf16),
            xTown=np.ascontiguousarray(own).astype(bf16),
            W1=Ws[0].astype(bf16),
            W2=Ws[1].astype(bf16),
            W3=Ws[2].astype(bf16),
            srcrow=metas[c]["srcrow"],
            srcrowG=metas[c]["srcrowG"],
            dstmod=metas[c]["dstmod"].astype(bf16),
            ewt=metas[c]["ew"].astype(bf16),
        )
        for li in range(3):
            m[f"asb{li + 1}"] = layer_params[li]["asb"].astype(bf16)
            m[f"adb{li + 1}"] = layer_params[li]["adb"].astype(bf16)
            m[f"ceb{li + 1}"] = layer_params[li]["ceb"].astype(bf16)
            m[f"bb{li + 1}"] = layer_params[li]["bb"]
        in_maps.append(m)

    results, _BENCH = _run_via_pjrt(nc, in_maps)
    LAST_RESULT = results

    out = np.empty((N, OUT), np.float32)
    allout = np.stack([results[c]["out"] for c in range(NCORES)])  # [8, NPAD, OUT]
    out[:] = allout[node_core, node_loc]
    return out.reshape(1, N, OUT)


# revision 11
# speedup vs baseline: 1.0250x; 1.0250x over previous
"""Distributed 3-layer GAT kernel for Trainium2 (8 NeuronCores).

Strategy (dst-sharded edges, node-sharded dense):
  - Destination nodes are assigned to 80 (core, block) bins by a greedy
    balance on in-degree, so every bin carries ~E/80 edges and the per-core
    gather row counts are uniform (the HBM row-gather at ~14 ns/row is the
    kernel's hard floor; balanced bins minimize the max-core row count).
  - Each core owns all edges whose destination bin lives on it, so the
    per-destination softmax needs no cross-core reduction.
  - Layer 1's dense projection (contraction dim = 96) is REPLICATED: every
    core computes the full 10240-row z table locally from the (tiny,
    replicated) input, eliminating layer 1's AllGather entirely.
  - Layers 2/3: each core computes z for its own rows, then chunked
    AllGathers replicate the table, overlapping the previous layer's edge
    processing.
  - Tables are [z fp8 | a_src bf16] rows (1040/770 bytes, u8 tensors +
    bitcast views): halves AllGather volume and HBM write traffic (less
    contention with the random gathers). The fp8 cast is a DVE copy at
    table-write time (once per row), and the gs multiply consumes fp8
    directly via a mixed-dtype DVE op -- no per-edge convert. (Casting
    during SWDGE DMA was tried and is pathologically slow.)
  - Edge chunks (128 edges) gather their source rows via indirect DMA into a
    per-block tile; one-hot(dst-slot) matmuls on the PE accumulate softmax
    denominators and scatter-add messages into PSUM. The alpha chain is
    packed per block ([128, M*H] ops) instead of per chunk.
  - Softmax is computed without the segment-max shift (logits are bounded
    for this model); the 1e-16 eps matches the reference.
"""

import sys

sys.path.insert(0, "/opt/trn_rl_repo")

import heapq

import numpy as np

# Problem constants (hardcoded per contract)
N = 10000
E = 160000
SEQ = 96
HID = 128
HEADS = 8
OUT = 768
HC = HID * HEADS  # 1024

NCORES = 8
NB = 10      # 128-node destination blocks per core
NPC = NB * 128  # padded nodes per core (1280)
NPAD = NPC
P = 128
NBINS = NCORES * NB
GR = 256     # rows per AllGather group (2 node blocks)

# table row layouts (bytes): [z fp8 (FO) | a_src bf16 (2H)]
ROWB = [HC + 2 * HEADS, HC + 2 * HEADS, OUT + 2]  # 1040, 1040, 770

LAST_RESULT = None


def _edge_prep(edge_index, edge_weight):
    """Balanced dst binning + per-core chunked edge arrays.

    Returns (MB, offs, CHT, metas, node_core, node_loc) where node_loc is the
    0..1279 row of each node within its core (block*128 + slot).
    """
    src = edge_index[0].astype(np.int64)
    dst = edge_index[1].astype(np.int64)
    indeg = np.bincount(dst, minlength=N)

    # greedy: highest-degree nodes first, into the least-loaded non-full bin
    order = np.argsort(-indeg, kind="stable")
    bin_cnt = np.zeros(NBINS, np.int64)
    node_bin = np.empty(N, np.int64)
    heap = [(0, b) for b in range(NBINS)]
    heapq.heapify(heap)
    for n in order:
        while True:
            load, b = heapq.heappop(heap)
            if bin_cnt[b] < P:
                break
        node_bin[n] = b
        bin_cnt[b] += 1
        if bin_cnt[b] < P:
            heapq.heappush(heap, (load + int(indeg[n]), b))
    node_core = node_bin // NB
    node_block = node_bin % NB
    node_slot = np.empty(N, np.int64)
    for b in range(NBINS):
        nodes = np.nonzero(node_bin == b)[0]
        node_slot[nodes] = np.arange(len(nodes))
    node_loc = node_block * P + node_slot

    # table row ids of each edge's source
    srank, sloc = node_core[src], node_loc[src]
    row_r = srank * NPAD + sloc                                   # rank-major (L1)
    row_g = (sloc // GR) * (NCORES * GR) + srank * GR + sloc % GR  # group-major (L2/3)

    dcore, dblock, dslot = node_core[dst], node_block[dst], node_slot[dst]

    cnt = np.zeros((NCORES, NB), np.int64)
    np.add.at(cnt, (dcore, dblock), 1)
    MB = np.maximum(1, (cnt.max(axis=0) + P - 1) // P).astype(np.int64)
    offs = np.concatenate([[0], np.cumsum(MB)]).astype(np.int64)
    CHT = int(MB.sum())

    metas = []
    for c in range(NCORES):
        srm = np.zeros((P, CHT), np.int32)
        srmG = np.zeros((P, CHT), np.int32)
        dm = np.full((P, CHT), 999.0, np.float32)
        ewm = np.zeros((P, CHT), np.float32)
        for b in range(NB):
            sel = np.nonzero((dcore == c) & (dblock == b))[0]
            k = len(sel)
            lanes = np.arange(k) % P
            cols = offs[b] + np.arange(k) // P
            srm[lanes, cols] = row_r[sel]
            srmG[lanes, cols] = row_g[sel]
            dm[lanes, cols] = dslot[sel].astype(np.float32)
            ewm[lanes, cols] = edge_weight[sel]
        metas.append(dict(srcrow=srm, srcrowG=srmG, dstmod=dm, ew=ewm))
    return MB, offs, CHT, metas, node_core, node_loc


def _build_program(MB, offs, CHT, sim_single_core=False):
    from concourse import bass, bacc, mybir, tile
    from concourse.masks import make_identity

    f32 = mybir.dt.float32
    bf = mybir.dt.bfloat16
    f8 = mybir.dt.float8e4
    u8 = mybir.dt.uint8
    i32 = mybir.dt.int32
    AT = mybir.ActivationFunctionType
    OP = mybir.AluOpType

    ndev = 1 if sim_single_core else NCORES
    nc = bacc.Bacc(None, target_bir_lowering=False, debug=False,
                   num_devices=ndev, num_swdge_queues=4)

    # ---------------- I/O ----------------
    xT_t = nc.dram_tensor("xT", [SEQ, NCORES * NPAD], bf, kind="ExternalInput")
    xTown_t = nc.dram_tensor("xTown", [SEQ, NPAD], bf, kind="ExternalInput")
    W_t = [
        nc.dram_tensor("W1", [SEQ, HC], bf, kind="ExternalInput"),
        nc.dram_tensor("W2", [HC, HC], bf, kind="ExternalInput"),
        nc.dram_tensor("W3", [HC, OUT], bf, kind="ExternalInput"),
    ]
    asb_t = [
        nc.dram_tensor("asb1", [P, HC], bf, kind="ExternalInput"),
        nc.dram_tensor("asb2", [P, HC], bf, kind="ExternalInput"),
        nc.dram_tensor("asb3", [P, OUT], bf, kind="ExternalInput"),
    ]
    adb_t = [
        nc.dram_tensor("adb1", [P, HC], bf, kind="ExternalInput"),
        nc.dram_tensor("adb2", [P, HC], bf, kind="ExternalInput"),
        nc.dram_tensor("adb3", [P, OUT], bf, kind="ExternalInput"),
    ]
    ceb_t = [
        nc.dram_tensor("ceb1", [P, HEADS], bf, kind="ExternalInput"),
        nc.dram_tensor("ceb2", [P, HEADS], bf, kind="ExternalInput"),
        nc.dram_tensor("ceb3", [P, 1], bf, kind="ExternalInput"),
    ]
    bb_t = [
        nc.dram_tensor("bb1", [P, HC], f32, kind="ExternalInput"),
        nc.dram_tensor("bb2", [P, HC], f32, kind="ExternalInput"),
        nc.dram_tensor("bb3", [P, OUT], f32, kind="ExternalInput"),
    ]
    srcrow_t = nc.dram_tensor("srcrow", [P, CHT], i32, kind="ExternalInput")
    srcrowG_t = nc.dram_tensor("srcrowG", [P, CHT], i32, kind="ExternalInput")
    dstmod_t = nc.dram_tensor("dstmod", [P, CHT], bf, kind="ExternalInput")
    ew_t = nc.dram_tensor("ewt", [P, CHT], bf, kind="ExternalInput")
    out_t = nc.dram_tensor("out", [NPAD, OUT], f32, kind="ExternalOutput")

    # layer configs: (K_in, FO, H, C, relu)
    LCFG = [
        (SEQ, HC, HEADS, HID, True),
        (HC, HC, HEADS, HID, True),
        (HC, OUT, 1, OUT, False),
    ]

    with tile.TileContext(nc) as tc:
        with (
            tc.tile_pool(name="const", bufs=1) as cpool,
            tc.tile_pool(name="dram", bufs=1, space="DRAM") as dpool,
            tc.tile_pool(name="work", bufs=2) as wpool,
            tc.tile_pool(name="gat", bufs=2) as gpool,
            tc.tile_pool(name="pbig", bufs=2, space="PSUM") as pbig,
            tc.tile_pool(name="psmall", bufs=2, space="PSUM") as psmall,
        ):
            # ---------------- constants ----------------
            ident = cpool.tile([P, P], bf, name="ident", tag="ident")
            make_identity(nc, ident[:])
            iota_i = cpool.tile([P, P], i32, name="iota_i", tag="iota_i")
            nc.gpsimd.iota(iota_i[:], pattern=[[1, P]], base=0, channel_multiplier=0)
            iota_f = cpool.tile([P, P], bf, name="iota_f", tag="iota_f")
            nc.vector.tensor_copy(iota_f[:], iota_i[:])

            xT_sb = cpool.tile([SEQ, NCORES * NPAD], bf, name="xT_sb", tag="xT_sb")
            nc.sync.dma_start(xT_sb[:], xT_t[:])
            xTown_sb = cpool.tile([SEQ, NPAD], bf, name="xTown_sb", tag="xTown_sb")
            nc.sync.dma_start(xTown_sb[:], xTown_t[:])

            srcrow_sb = cpool.tile([P, CHT], i32, name="srcrow_sb", tag="srcrow_sb")
            nc.sync.dma_start(srcrow_sb[:], srcrow_t[:])
            srcrowG_sb = cpool.tile([P, CHT], i32, name="srcrowG_sb", tag="srcrowG_sb")
            nc.sync.dma_start(srcrowG_sb[:], srcrowG_t[:])
            dstmod_sb = cpool.tile([P, CHT], bf, name="dstmod_sb", tag="dstmod_sb")
            nc.sync.dma_start(dstmod_sb[:], dstmod_t[:])
            ew_sb = cpool.tile([P, CHT], bf, name="ew_sb", tag="ew_sb")
            nc.sync.dma_start(ew_sb[:], ew_t[:])

            # ---------------- internal DRAM ----------------
            ci, tb = [None], []
            for li in range(3):
                tb.append(
                    dpool.tile([NCORES * NPAD, ROWB[li]], u8,
                               name=f"tb{li}", tag=f"tb{li}")
                )
            for li in (1, 2):
                ci.append(
                    dpool.tile([NPAD, ROWB[li]], u8, name=f"ci{li}", tag=f"ci{li}")
                )

            def stats_from_sbuf(zb, coef, H, C, FO):
                tmp = wpool.tile([P, FO], bf, name="tmp", tag="stat_tmp")
                nc.vector.tensor_mul(tmp[:], zb[:], coef[:])
                red = wpool.tile([P, H], f32, name="red", tag="red", bufs=4)
                nc.vector.tensor_reduce(
                    out=red[:],
                    in_=tmp[:].rearrange("p (h c) -> p h c", c=C),
                    axis=mybir.AxisListType.X,
                    op=OP.add,
                )
                redb = wpool.tile([P, H], bf, name="redb", tag="redb", bufs=4)
                nc.vector.tensor_copy(redb[:], red[:])
                return redb

            def load_layer_params(li, H, FO, K_in):
                nk = (K_in + P - 1) // P
                W_l = []
                for kc in range(nk):
                    k0, k1 = kc * P, min(K_in, kc * P + P)
                    wt = cpool.tile([k1 - k0, FO], bf, name="wt", tag=f"w_{kc}")
                    nc.sync.dma_start(wt[:], W_t[li][k0:k1, :])
                    W_l.append(wt)
                a_s_b = cpool.tile([P, FO], bf, name="a_s_b", tag="asb")
                nc.sync.dma_start(a_s_b[:], asb_t[li][:])
                a_d_b = cpool.tile([P, FO], bf, name="a_d_b", tag="adb")
                nc.sync.dma_start(a_d_b[:], adb_t[li][:])
                ce_b = cpool.tile([P, H], bf, name="ce_b", tag="ceb", bufs=2)
                nc.sync.dma_start(ce_b[:], ceb_t[li][:])
                bb_b = cpool.tile([P, FO], f32, name="bb_b", tag="bb", bufs=2)
                nc.sync.dma_start(bb_b[:], bb_t[li][:])
                return W_l, a_s_b, a_d_b, ce_b, bb_b

            # ---------------- layer 1: replicated full-table sweep ----------
            W1_l, as1_b, ad1_b, ce1_b, bb1_b = load_layer_params(0, HEADS, HC, SEQ)
            adall1 = wpool.tile([P, NB * HEADS], bf, name="adall1", tag="adall", bufs=2)
            for gb in range(NCORES * NB):
                z_ps = pbig.tile([P, HC], f32, name="z_ps", tag="big")
                for j in range(2):
                    j0, j1 = j * 512, (j + 1) * 512
                    nc.tensor.matmul(
                        out=z_ps[:, j0:j1],
                        lhsT=xT_sb[:, gb * P : (gb + 1) * P],
                        rhs=W1_l[0][:, j0:j1],
                        start=True, stop=True,
                    )
                zb = wpool.tile([P, HC], bf, name="zb", tag="zb")
                nc.vector.tensor_copy(zb[:], z_ps[:])
                z8 = wpool.tile([P, HC], f8, name="z8", tag="z8")
                nc.vector.tensor_copy(z8[:], z_ps[:])
                nc.sync.dma_start(
                    tb[0][gb * P : (gb + 1) * P, 0:HC].bitcast(f8), z8[:]
                )
                redb = stats_from_sbuf(zb, as1_b, HEADS, HID, HC)
                nc.sync.dma_start(
                    tb[0][gb * P : (gb + 1) * P, HC : HC + 2 * HEADS].bitcast(bf),
                    redb[:],
                )
            # own-rows mini-sweep for a_dst (rank-independent: uses own input)
            for nb in range(NB):
                z_ps = pbig.tile([P, HC], f32, name="z_ps", tag="big")
                for j in range(2):
                    j0, j1 = j * 512, (j + 1) * 512
                    nc.tensor.matmul(
                        out=z_ps[:, j0:j1],
                        lhsT=xTown_sb[:, nb * P : (nb + 1) * P],
                        rhs=W1_l[0][:, j0:j1],
                        start=True, stop=True,
                    )
                zb = wpool.tile([P, HC], bf, name="zb", tag="zb")
                nc.vector.tensor_copy(zb[:], z_ps[:])
                redb = stats_from_sbuf(zb, ad1_b, HEADS, HID, HC)
                nc.vector.tensor_copy(
                    adall1[:, nb * HEADS : (nb + 1) * HEADS], redb[:]
                )

            # ---------------- helpers ------------------------------------
            def dense_block(li, nb, f_in, W_l, a_s_b, a_d_b, adall):
                K_in, FO, H, C, relu = LCFG[li]
                nk = (K_in + P - 1) // P
                nj = (FO + 511) // 512
                lhsTs = []
                for kc in range(nk):
                    tr_ps = psmall.tile([P, P], bf, name="tr_ps", tag="tr", bufs=1)
                    nc.tensor.transpose(
                        out=tr_ps[:],
                        in_=f_in[:, kc * P : (kc + 1) * P],
                        identity=ident[:],
                    )
                    lt = wpool.tile([P, P], bf, name="lt", tag="lt", bufs=10)
                    nc.vector.tensor_copy(lt[:], tr_ps[:])
                    lhsTs.append(lt[:])
                z_ps = pbig.tile([P, FO], f32, name="z_ps", tag="big")
                for j in range(nj):
                    j0, j1 = j * 512, min(FO, (j + 1) * 512)
                    for kc in range(nk):
                        nc.tensor.matmul(
                            out=z_ps[:, j0:j1],
                            lhsT=lhsTs[kc],
                            rhs=W_l[kc][:, j0:j1],
                            start=(kc == 0),
                            stop=(kc == nk - 1),
                        )
                zb = wpool.tile([P, FO], bf, name="zb", tag="zb")
                nc.vector.tensor_copy(zb[:], z_ps[:])
                z8 = wpool.tile([P, FO], f8, name="z8", tag="z8")
                nc.vector.tensor_copy(z8[:], z_ps[:])
                nc.sync.dma_start(
                    ci[li][nb * P : (nb + 1) * P, 0:FO].bitcast(f8), z8[:]
                )
                redb = stats_from_sbuf(zb, a_s_b, H, C, FO)
                nc.sync.dma_start(
                    ci[li][nb * P : (nb + 1) * P, FO : FO + 2 * H].bitcast(bf),
                    redb[:],
                )
                redd = stats_from_sbuf(zb, a_d_b, H, C, FO)
                nc.vector.tensor_copy(adall[:, nb * H : (nb + 1) * H], redd[:])

            def ag(li, r0, r1):
                g8 = (r0 // GR) * NCORES * GR
                if sim_single_core:
                    nc.gpsimd.dma_start(
                        tb[li][g8 : g8 + (r1 - r0), :], ci[li][r0:r1, :]
                    )
                else:
                    nc.gpsimd.collective_compute(
                        "AllGather",
                        OP.bypass,
                        replica_groups=[list(range(NCORES))],
                        ins=[ci[li][r0:r1, :].opt()],
                        outs=[tb[li][g8 : g8 + NCORES * (r1 - r0), :].opt()],
                    )

            def agg_block(li, nb, srcrow, ce_b, bb_b, adall):
                K_in, FO, H, C, relu = LCFG[li]
                RB = ROWB[li]
                nj = (FO + 511) // 512
                M = int(MB[nb])
                c0 = int(offs[nb])

                g_all = gpool.tile([P, M * RB], u8, name="g_all", tag="g", bufs=2)
                for m in range(M):
                    nc.gpsimd.indirect_dma_start(
                        out=g_all[:, m * RB : (m + 1) * RB],
                        out_offset=None,
                        in_=tb[li][:],
                        in_offset=bass.IndirectOffsetOnAxis(
                            ap=srcrow[:, c0 + m : c0 + m + 1], axis=0
                        ),
                    )
                g3 = g_all[:].rearrange("p (m r) -> p m r", r=RB)

                oh_all = gpool.tile([P, M * P], bf, name="oh_all", tag="oh", bufs=2)
                nc.vector.tensor_tensor(
                    out=oh_all[:].rearrange("p (m j) -> p m j", j=P),
                    in0=dstmod_sb[:, c0 : c0 + M].unsqueeze(2).to_broadcast([P, M, P]),
                    in1=iota_f[:].unsqueeze(1).to_broadcast([P, M, P]),
                    op=OP.is_equal,
                )

                ad_ps = psmall.tile([P, M * H], f32, name="ad_ps", tag="adp", bufs=2)
                for m in range(M):
                    trB = psmall.tile([P, P], bf, name="trB", tag="tr", bufs=1)
                    nc.tensor.transpose(
                        out=trB[:], in_=oh_all[:, m * P : (m + 1) * P],
                        identity=ident[:],
                    )
                    ohB = wpool.tile([P, P], bf, name="ohB", tag="ohB", bufs=3)
                    nc.vector.tensor_copy(ohB[:], trB[:])
                    nc.tensor.matmul(
                        out=ad_ps[:, m * H : (m + 1) * H],
                        lhsT=ohB[:],
                        rhs=adall[:, nb * H : (nb + 1) * H],
                        start=True, stop=True,
                    )

                # packed alpha chain over [P, M*H]
                asrc_ap = g3[:, :, FO : FO + 2 * H].bitcast(bf)
                ewce = wpool.tile([P, M * H], bf, name="ewce", tag="ewce")
                nc.vector.tensor_tensor(
                    out=ewce[:].rearrange("p (m h) -> p m h", h=H),
                    in0=ew_sb[:, c0 : c0 + M].unsqueeze(2).to_broadcast([P, M, H]),
                    in1=ce_b[:].unsqueeze(1).to_broadcast([P, M, H]),
                    op=OP.mult,
                )
                al = wpool.tile([P, M * H], bf, name="al", tag="al")
                nc.vector.tensor_tensor(
                    out=al[:].rearrange("p (m h) -> p m h", h=H),
                    in0=ewce[:].rearrange("p (m h) -> p m h", h=H),
                    in1=asrc_ap,
                    op=OP.add,
                )
                al2 = wpool.tile([P, M * H], bf, name="al2", tag="al2")
                nc.vector.tensor_add(al2[:], al[:], ad_ps[:])
                al3 = wpool.tile([P, M * H], bf, name="al3", tag="al3")
                nc.vector.scalar_tensor_tensor(
                    out=al3[:], in0=al2[:], scalar=0.2, in1=al2[:],
                    op0=OP.mult, op1=OP.max,
                )
                ex_all = wpool.tile([P, M * H], bf, name="ex_all", tag="ex", bufs=2)
                nc.scalar.activation(out=ex_all[:], in_=al3[:], func=AT.Exp)

                # weighted scatter + denominator
                agg_ps = pbig.tile([P, FO], f32, name="agg_ps", tag="big")
                den_ps = psmall.tile([P, H], f32, name="den_ps", tag="den", bufs=1)
                for m in range(M):
                    gs = wpool.tile([P, FO], bf, name="gs", tag="gs", bufs=3)
                    nc.vector.tensor_tensor(
                        out=gs[:].rearrange("p (h c) -> p h c", c=C),
                        in0=g3[:, m, 0:FO].bitcast(f8)
                            .rearrange("p (h c) -> p h c", c=C),
                        in1=ex_all[:, m * H : (m + 1) * H]
                            .unsqueeze(2).to_broadcast([P, H, C]),
                        op=OP.mult,
                    )
                    nc.tensor.matmul(
                        out=den_ps[:],
                        lhsT=oh_all[:, m * P : (m + 1) * P],
                        rhs=ex_all[:, m * H : (m + 1) * H],
                        start=(m == 0), stop=(m == M - 1),
                    )
                    for j in range(nj):
                        j0, j1 = j * 512, min(FO, (j + 1) * 512)
                        nc.tensor.matmul(
                            out=agg_ps[:, j0:j1],
                            lhsT=oh_all[:, m * P : (m + 1) * P],
                            rhs=gs[:, j0:j1],
                            start=(m == 0), stop=(m == M - 1),
                        )

                den_sb = wpool.tile([P, H], f32, name="den_sb", tag="den_sb")
                nc.vector.tensor_scalar_add(den_sb[:], den_ps[:], 1e-16)
                rec = wpool.tile([P, H], f32, name="rec", tag="rec")
                nc.vector.reciprocal(rec[:], den_sb[:])
                o1 = wpool.tile([P, FO], f32, name="o1", tag="o1")
                nc.vector.tensor_tensor(
                    out=o1[:].rearrange("p (h c) -> p h c", c=C),
                    in0=agg_ps[:].rearrange("p (h c) -> p h c", c=C),
                    in1=rec[:].unsqueeze(2).to_broadcast([P, H, C]),
                    op=OP.mult,
                )
                o2 = wpool.tile([P, FO], f32, name="o2", tag="o2")
                nc.vector.tensor_add(o2[:], o1[:], bb_b[:])
                if relu:
                    fnew = wpool.tile([P, FO], bf, name="fnew", tag="fnew")
                    nc.scalar.activation(out=fnew[:], in_=o2[:], func=AT.Relu)
                    return fnew
                nc.sync.dma_start(out_t[nb * P : (nb + 1) * P, :], o2[:])
                return None

            # ---------------- layers 2/3 pipelined over blocks -------------
            prevctx = (0, srcrow_sb, ce1_b, bb1_b, adall1)
            for li in (1, 2):
                K_in, FO, H, C, relu = LCFG[li]
                W_l, a_s_b, a_d_b, ce_b, bb_b = load_layer_params(li, H, FO, K_in)
                adall = wpool.tile([P, NB * H], bf, name="adall", tag="adall", bufs=2)

                for nb in range(NB):
                    pli, psrc, pce, pbb, pad = prevctx
                    fnew = agg_block(pli, nb, psrc, pce, pbb, pad)
                    dense_block(li, nb, fnew, W_l, a_s_b, a_d_b, adall)
                    if nb % 2 == 1:
                        g = (nb - 1) // 2
                        ag(li, g * GR, (g + 1) * GR)

                prevctx = (li, srcrowG_sb, ce_b, bb_b, adall)

            pli, psrc, pce, pbb, pad = prevctx
            for nb in range(NB):
                agg_block(pli, nb, psrc, pce, pbb, pad)

    nc.finalize()
    return nc


def _run_via_pjrt(nc, in_maps):
    """Multi-core run via PJRT without output donation, returning a reusable
    compiled callable for steady-state timing."""
    import jax
    import numpy as _np
    from jax.sharding import Mesh, PartitionSpec
    from jax.experimental.shard_map import shard_map
    from concourse import bass2jax, mybir

    bass2jax.install_neuronx_cc_hook()

    partition_name = nc.partition_id_tensor.name if nc.partition_id_tensor else None
    in_names, out_names, out_avals, zero_outs = [], [], [], []
    for alloc in nc.m.functions[0].allocations:
        if not isinstance(alloc, mybir.MemoryLocationSet):
            continue
        name = alloc.memorylocations[0].name
        if alloc.kind == "ExternalInput":
            if name != partition_name:
                in_names.append(name)
        elif alloc.kind == "ExternalOutput":
            shape = tuple(alloc.tensor_shape)
            dtype = mybir.dt.np(alloc.dtype)
            out_names.append(name)
            out_avals.append(jax.core.ShapedArray(shape, dtype))
            zero_outs.append(_np.zeros(shape, dtype))
    n_params = len(in_names)
    all_in_names = in_names + out_names
    if partition_name is not None:
        all_in_names = all_in_names + [partition_name]

    def _body(*args):
        operands = list(args)
        if partition_name is not None:
            operands.append(bass2jax.partition_id_tensor())
        outs = bass2jax._bass_exec_p.bind(
            *operands,
            out_avals=tuple(out_avals),
            in_names=tuple(all_in_names),
            out_names=tuple(out_names),
            lowering_input_output_aliases=(),
            sim_require_finite=True,
            sim_require_nnan=True,
            nc=nc,
        )
        return tuple(outs)

    n = len(in_maps)
    devices = jax.devices()[:n]
    mesh = Mesh(_np.asarray(devices), ("core",))
    specs = (PartitionSpec("core"),) * (n_params + len(out_names))
    out_specs = (PartitionSpec("core"),) * len(out_names)
    fn = jax.jit(
        shard_map(_body, mesh=mesh, in_specs=specs, out_specs=out_specs,
                  check_rep=False),
        keep_unused=True,
    )
    concat_in = [
        _np.concatenate([_np.asarray(in_maps[c][k]) for c in range(n)], axis=0)
        for k in in_names
    ] + [
        _np.zeros((n * z.shape[0], *z.shape[1:]), z.dtype) for z in zero_outs
    ]
    sharding = jax.sharding.NamedSharding(mesh, PartitionSpec("core"))
    dev_in = [jax.device_put(a, sharding) for a in concat_in]
    out_arrs = fn(*dev_in)
    jax.block_until_ready(out_arrs)
    results = [
        {
            name: _np.asarray(out_arrs[i]).reshape(n, *out_avals[i].shape)[c]
            for i, name in enumerate(out_names)
        }
        for c in range(n)
    ]
    return results, (fn, dev_in)


_BENCH = None


def bench(n_iters=20):
    """Median wall time (ns) of one steady-state invocation of the compiled
    8-core executable with device-resident inputs."""
    import jax, time
    assert _BENCH is not None, "call kernel() first"
    fn, dev_in = _BENCH
    jax.block_until_ready(fn(*dev_in))  # warm
    times = []
    for _ in range(n_iters):
        t0 = time.perf_counter()
        jax.block_until_ready(fn(*dev_in))
        t1 = time.perf_counter()
        times.append(t1 - t0)
    times.sort()
    return times[len(times) // 2] * 1e9


def kernel(**inputs):
    global LAST_RESULT, _BENCH

    x = np.asarray(inputs["x"], np.float32)
    edge_index = np.asarray(inputs["edge_index"], np.int32)
    edge_weight = np.asarray(inputs["edge_weight"], np.float32)

    MB, offs, CHT, metas, node_core, node_loc = _edge_prep(edge_index, edge_weight)
    nc = _build_program(MB, offs, CHT)

    # xT in global table-row (rank-major) order, pad slots zero
    xT = np.zeros((SEQ, NCORES * NPAD), np.float32)
    xT[:, node_core * NPAD + node_loc] = x[0]

    def bcast(v):
        v = np.asarray(v, np.float32).reshape(1, -1)
        return np.ascontiguousarray(np.repeat(v, P, axis=0))

    Ws = [np.asarray(inputs[k], np.float32) for k in ("W1", "W2", "W3")]
    layer_params = []
    for li, (aek, wek, ask, adk, bk, H, C) in enumerate(
        (
            ("ae1", "We1", "as1", "ad1", "b1", HEADS, HID),
            ("ae2", "We2", "as2", "ad2", "b2", HEADS, HID),
            ("ae3", "We3", "as3", "ad3", "b3", 1, OUT),
        )
    ):
        ae = np.asarray(inputs[aek], np.float32)
        We = np.asarray(inputs[wek], np.float32)
        ce = np.array(
            [We[0, h * C : (h + 1) * C] @ ae[h] for h in range(H)], np.float32
        )
        layer_params.append(
            dict(
                asb=bcast(np.asarray(inputs[ask], np.float32).reshape(-1)),
                adb=bcast(np.asarray(inputs[adk], np.float32).reshape(-1)),
                ceb=bcast(ce),
                bb=bcast(np.asarray(inputs[bk], np.float32)),
            )
        )

    import ml_dtypes

    bf16 = ml_dtypes.bfloat16
    in_maps = []
    for c in range(NCORES):
        own = xT[:, c * NPAD : (c + 1) * NPAD]
        m = dict(
            xT=xT.astype(bf16),
            xTown=np.ascontiguousarray(own).astype(bf16),
            W1=Ws[0].astype(bf16),
            W2=Ws[1].astype(bf16),
            W3=Ws[2].astype(bf16),
            srcrow=metas[c]["srcrow"],
            srcrowG=metas[c]["srcrowG"],
            dstmod=metas[c]["dstmod"].astype(bf16),
            ewt=metas[c]["ew"].astype(bf16),
        )
        for li in range(3):
            m[f"asb{li + 1}"] = layer_params[li]["asb"].astype(bf16)
            m[f"adb{li + 1}"] = layer_params[li]["adb"].astype(bf16)
            m[f"ceb{li + 1}"] = layer_params[li]["ceb"].astype(bf16)
            m[f"bb{li + 1}"] = layer_params[li]["bb"]
        in_maps.append(m)

    results, _BENCH = _run_via_pjrt(nc, in_maps)
    LAST_RESULT = results

    out = np.empty((N, OUT), np.float32)
    allout = np.stack([results[c]["out"] for c in range(NCORES)])  # [8, NPAD, OUT]
    out[:] = allout[node_core, node_loc]
    return out.reshape(1, N, OUT)
